# revision 18
# baseline (speedup 1.0000x reference)
"""CrossLinear attention kernel for Trainium2 (8 NeuronCores, data-parallel over batch).

Computes, per batch element b:
    scores = x_b @ x_b^T            [T, T]
    scores[mask] = -inf
    attn = softmax(scores, axis=-1)
    xx = x_b @ W                    [T, C]
    out_b = attn @ xx               [T, C]

with B=8, T=2048, C=1024 (fp32).  One batch element per NeuronCore.

Design notes (v2 — symmetric scores):
  - All big matmuls use float32r operands -> 1 cycle/row on the PE when the
    moving dim is >=256 (plain fp32 is 4 cycles/row).  End-to-end rel err
    stays ~1e-3.
  - scores = x x^T is SYMMETRIC: stripe i only matmuls blocks (i, j>=i);
    blocks (i, j<i) are PE-transposed (1.5 cyc/row, ~5x cheaper than the
    matmul they replace) from raw (pre-mask) 128x128 blocks saved by
    earlier stripes.  Raw blocks are copied PSUM->SBUF on the scalar
    engine before the DVE mask-fill of the same PSUM region; a 64-slot
    interval-colored save pool (exact peak occupancy) holds them.
  - x is transposed once on the PE into xT [C, T] (f32r input via ACT cast
    so the transposes run 1.5 cyc/row); both score-matmul operands and the
    x@W lhsT come from xT.
  - The attn @ xx GEMM runs in bf16: xx is stored bf16 and the attn
    transposes (f32r on the PE) are cast to bf16 on the psum->sbuf copy.
    PSUM accumulates fp32, so only input rounding (~0.2%) is added.
  - Masking via copy_predicated (mask u8, -1e9 fill) per 512-col bank,
    interleaved with the score work; softmax: per-bank DVE row max ->
    combine (negated) -> ACT exp with per-partition bias and accumulated
    row sum -> 1/sum folded into the final output scale.
  - Phase ordering: x DMAs are issued before W so the PE starts early; the
    x-transposes and x@W matmuls interleave; scores(0) is emitted right
    after the last x@W matmul so the PE never drains at the phase
    boundary.  PSUM pools are staged (psA+psXX = 6 banks, then psS joins
    at 4+4 = 8, then psT/psO reuse psXX's banks).
  - This container's walrus accepts at most ONE sync-wait per instruction;
    _split_sync_waits hoists extras onto single-wait NoOps.
"""

import sys

if "/opt/trn_rl_repo" not in sys.path:
    sys.path.insert(0, "/opt/trn_rl_repo")

from contextlib import ExitStack

import numpy as np

import concourse.bass as bass
import concourse.mybir as mybir
import concourse.tile as tile
from concourse import bass_utils
from concourse.bass import ds, ts
from concourse.masks import make_identity

B, T, C = 8, 2048, 1024
P = 128                 # partition block
NT = T // P             # 16 row blocks (stripes)
NKC = C // P            # 8 contraction chunks over C
NKS = T // P            # 16 contraction chunks over T (for attn @ xx)
NSLOTS = 64             # saved raw score blocks (exact interval-coloring peak)
NEG_BIG = -1.0e9

F32 = mybir.dt.float32
F32R = mybir.dt.float32r
BF16 = mybir.dt.bfloat16
U8 = mybir.dt.uint8


def _span_chunks(start_col: int):
    """Bank-aligned (512-col) chunk widths covering [start_col, T)."""
    chunks = []
    col = start_col
    while col < T:
        bank_end = (col // 512 + 1) * 512
        w = min(bank_end, T) - col
        chunks.append((col, w))
        col += w
    return chunks


def build_bass():
    nc = bass.Bass(
        trn_type="TRN2",
        target_bir_lowering=False,
        debug=False,
        enable_asserts=False,
        num_devices=8,
    )
    x_d = nc.dram_tensor("x", [T, C], F32, kind="ExternalInput").ap()
    m_d = nc.dram_tensor("mask", [T, T], U8, kind="ExternalInput").ap()
    w_d = nc.dram_tensor("W", [C, C], F32, kind="ExternalInput").ap()
    o_d = nc.dram_tensor("out", [T, C], F32, kind="ExternalOutput").ap()

    with tile.TileContext(nc) as tc:
        _kernel_body(nc, tc, x_d, m_d, w_d, o_d)
    return nc


def _kernel_body(nc, tc, x_d, m_d, w_d, o_d):
    with ExitStack() as big:
        const = big.enter_context(tc.tile_pool(name="const", bufs=1))
        ident = const.tile([P, P], F32)
        make_identity(nc, ident[:])
        ident_r = const.tile([P, P], F32R)
        nc.vector.tensor_copy(ident_r[:], ident[:])
        negbig = const.tile([P, 512], F32)
        nc.gpsimd.memset(negbig[:], NEG_BIG)

        persist = big.enter_context(tc.tile_pool(name="persist", bufs=1))
        xT = persist.tile([P, NKC, T], F32R)   # xT[p, k, t] = x[t, k*128+p]
        xx = persist.tile([P, NT, C], BF16)    # xx[p, i, d] = (x@W)[i*128+p, d]
        # raw upper-triangle score blocks, producer-contiguous: stripe i's
        # saved span (cols 128(i+1)..T) lives at block offset sbase[i], so
        # one batched ACT copy per 512-col bank saves it.  block (j,i)
        # (consumed by stripe j>i) sits at sbase[i] + (j-i-1).
        sbase = [0]
        for i in range(1, NT):
            sbase.append(sbase[-1] + (NT - 1 - i + 1))
        save = persist.tile([P, sbase[-1] * P], F32R)

        def save_cols(j, i):
            """column range in `save` holding raw block (j, i), j > i."""
            return ds((sbase[i] + (j - i - 1)) * P, P)

        # W (f32r, 32KB/partition) is dead once the last x@W matmul ran and
        # `save` is first written in phase B -- alias W onto save's first
        # 8K columns; the framework's WAR tracking orders the reuse.
        def wv(k, h):
            return save[:, ds(k * C + h * 512, 512)]

        # ---- Phase A: transpose x into xT; compute xx = x @ W ----
        # f32r is raw fp32 bytes (the TF32-style rounding happens inside the
        # PE), so x and W DMA straight into f32r tiles via bitcast views --
        # no cast instructions, and the PE can start as soon as x[0] lands.
        phA = ExitStack()
        xload = phA.enter_context(tc.tile_pool(name="xload", bufs=3))
        psXX = phA.enter_context(tc.tile_pool(name="psXX", bufs=2, space="PSUM"))
        psA = phA.enter_context(tc.tile_pool(name="psA", bufs=2, space="PSUM"))

        xts = {}

        def xdma(i):
            t = xload.tile([P, C], F32R, tag="xt")
            nc.sync.dma_start(t[:], x_d[ts(i, P), :].bitcast(F32R))
            xts[i] = t

        def xpose_c(i):
            xr = xts.pop(i)
            for g in range(2):
                pt = psA.tile([P, 4, P], F32R, tag="pt")
                for j in range(4):
                    k = g * 4 + j
                    nc.tensor.transpose(
                        pt[:, j, :], xr[:, ds(k * P, P)], ident_r[:]
                    )
                nc.vector.tensor_copy(
                    xT[:, ds(g * 4, 4), ds(i * P, P)], pt[:]
                )

        def xxmm(i):
            po = psXX.tile([P, C], F32, tag="po1")
            for k in range(NKC):
                for h in range(2):
                    nc.tensor.matmul(
                        po[:, ds(h * 512, 512)],
                        lhsT=xT[:, k, ds(i * P, P)],
                        rhs=wv(k, h),
                        start=(k == 0),
                        stop=(k == NKC - 1),
                    )
            nc.vector.tensor_copy(xx[:, i, :], po[:])

        # x DMAs first so the PE can start early; W loads overlap.
        xdma(0)
        xdma(1)
        xdma(2)
        for k in range(NKC):
            nc.sync.dma_start(
                save[:, ds(k * C, C)], w_d[ts(k, P), :].bitcast(F32R)
            )

        xpose_c(0)
        for i in range(1, NT):
            xpose_c(i)
            if i + 2 < NT:
                xdma(i + 2)
            xxmm(i - 1)
        xxmm(NT - 1)

        # release phase-A SBUF + PSUM so phase B can reuse the space; the
        # PE queue still flows straight from xx(15) into scores(0)
        phA.close()

        # ---- Phase B: attention main loop over row stripes ----
        with tc.tile_pool(name="maskp", bufs=2) as maskp, \
             tc.tile_pool(name="ppool", bufs=2) as ppool, \
             tc.tile_pool(name="ptpool", bufs=2) as ptpool, \
             tc.tile_pool(name="opool", bufs=2) as opool, \
             tc.tile_pool(name="stats", bufs=4) as stats, \
             tc.tile_pool(name="psS", bufs=1, space="PSUM") as psS, \
             tc.tile_pool(name="psT", bufs=2, space="PSUM") as psT, \
             tc.tile_pool(name="psO", bufs=1, space="PSUM") as psO:

            def scores(i):
                """Score stripe i, emitted bank-by-bank (512 cols) so the DVE
                mask/rowmax chain and the ACT raw-block save of bank n overlap
                the PE work of banks n+1..: per bank emit [transposes of saved
                raw blocks (j<i)] [matmul chunk (j>=i)] [batched raw save]
                [mask-fill] [row max]."""
                msk = maskp.tile([P, T], U8, tag="mask")
                nc.sync.dma_start(msk[:], m_d[ts(i, P), :])
                ps = psS.tile([P, T], F32, tag="scores")
                maxpart = stats.tile([P, 4], F32, tag="maxpart")
                for n in range(4):
                    sl = ds(n * 512, 512)
                    b0, b1 = n * 4, n * 4 + 4   # 128-col blocks of this bank
                    # left part: PE-transpose saved raw blocks
                    for j in range(b0, min(b1, i)):
                        nc.tensor.transpose(
                            ps[:, ds(j * P, P)].bitcast(F32R),
                            save[:, save_cols(i, j)],
                            ident_r[:],
                        )
                    # right part: one bank-aligned matmul chunk
                    mm0 = max(b0 * P, i * P)
                    w = (b1 * P) - mm0
                    if w > 0:
                        msl = ds(mm0, w)
                        for k in range(NKC):
                            nc.tensor.matmul(
                                ps[:, msl],
                                lhsT=xT[:, k, ds(i * P, P)],
                                rhs=xT[:, k, msl],
                                start=(k == 0),
                                stop=(k == NKC - 1),
                            )
                    # batched raw save of this bank's j>i blocks (pre-mask)
                    s0 = max(b0, i + 1)
                    if s0 < b1:
                        nw = (b1 - s0) * P
                        nc.scalar.copy(
                            save[:, ds((sbase[i] + s0 - i - 1) * P, nw)],
                            ps[:, ds(s0 * P, nw)],
                        )
                    # mask + row max (DVE)
                    nc.vector.copy_predicated(ps[:, sl], msk[:, sl], negbig[:])
                    nc.vector.reduce_max(
                        maxpart[:, ds(n, 1)], ps[:, sl],
                        axis=mybir.AxisListType.X,
                    )
                return ps, maxpart

            def softmax(i, ps, maxpart):
                negmax = stats.tile([P, 1], F32, tag="negmax")
                nc.vector.reduce_max(
                    negmax[:], maxpart[:], axis=mybir.AxisListType.X, negate=True
                )
                # exp in two halves: pv_transpose groups 0-1 and the next
                # stripe's left-half PSUM writes only wait on exp half 0,
                # shaving ~1us off the per-stripe critical path
                rowsums = stats.tile([P, 2], F32, tag="rowsums")
                p_i = ppool.tile([P, T], F32R, tag="p")
                for g in range(2):
                    sl = ds(g * 1024, 1024)
                    nc.scalar.activation(
                        p_i[:, sl],
                        ps[:, sl],
                        mybir.ActivationFunctionType.Exp,
                        bias=negmax[:],
                        scale=1.0,
                        accum_out=rowsums[:, ds(g, 1)],
                    )
                rowsum = stats.tile([P, 1], F32, tag="rowsum")
                nc.vector.reduce_sum(
                    rowsum[:], rowsums[:], axis=mybir.AxisListType.X
                )
                recip = stats.tile([P, 1], F32, tag="recip")
                nc.vector.reciprocal(recip[:], rowsum[:])
                return p_i, recip

            def pv_transpose(i, p_i):
                """PE-transpose attn weights (f32r); psum->sbuf casts to
                bf16 on the scalar engine."""
                pT = ptpool.tile([P, NKS, P], BF16, tag="pT")
                for g in range(4):
                    pt_ps = psT.tile([P, 4, P], F32R, tag="ptps")
                    for j in range(4):
                        s = g * 4 + j
                        nc.tensor.transpose(
                            pt_ps[:, j, :], p_i[:, ds(s * P, P)], ident_r[:]
                        )
                    nc.scalar.copy(pT[:, ds(g * 4, 4), :], pt_ps[:])
                return pT

            def pv_out(i, pT, recip):
                po = psO.tile([P, C], F32, tag="po2")
                for s in range(NKS):
                    for h in range(2):
                        nc.tensor.matmul(
                            po[:, ds(h * 512, 512)],
                            lhsT=pT[:, s, :],
                            rhs=xx[:, s, ds(h * 512, 512)],
                            start=(s == 0),
                            stop=(s == NKS - 1),
                        )
                out_t = opool.tile([P, C], F32, tag="out")
                # the scale sits off the critical path (only the out DMA
                # consumes it); deprioritize it so the scheduler doesn't
                # slot it ahead of exp(i+1) on ACT or the mask chain on DVE
                with tc.high_priority(offset=-600):
                    nc.scalar.mul(out_t[:], po[:], recip[:])
                nc.sync.dma_start(o_d[ts(i, P), :], out_t[:])

            # software pipeline; engine-queue order is the point
            sc = scores(0)
            sm = softmax(0, *sc)
            for i in range(NT):
                nxt_sc = scores(i + 1) if i + 1 < NT else None
                pT = pv_transpose(i, sm[0])
                nxt_sm = softmax(i + 1, *nxt_sc) if nxt_sc else None
                pv_out(i, pT, sm[1])
                sm = nxt_sm


def _split_sync_waits(nc, limit: int = 1):
    """The walrus build in this container rejects instructions with more than
    one sync-wait command.  Hoist excess waits onto preceding single-wait
    NoOps on the same engine (waits execute in order before the original
    instruction, so semantics are preserved)."""
    n_new = 0
    for fn in nc.m.functions:
        for blk in fn.blocks:
            new_insts = []
            for inst in blk.instructions:
                si = inst.sync_info
                if si and si.on_wait and len(si.on_wait) > limit:
                    waits = list(si.on_wait)
                    extra, keep = waits[:-limit], waits[-limit:]
                    for w in extra:
                        nop = mybir.InstNoOp(
                            name=f"{inst.name}-wsplit{n_new}", ins=[], outs=[]
                        )
                        n_new += 1
                        nop.engine = inst.engine
                        nop.sync_info = mybir.SyncInfo(on_wait=[w], on_update=[])
                        new_insts.append(nop)
                    si.on_wait[:] = keep
                new_insts.append(inst)
            blk.instructions[:] = new_insts
    return n_new


_NC_CACHE = None


def _get_nc():
    global _NC_CACHE
    if _NC_CACHE is None:
        nc = build_bass()
        _split_sync_waits(nc, limit=1)
        _NC_CACHE = nc
    return _NC_CACHE


def run(inputs: dict, trace: bool = False, tmpdir: str | None = None):
    """Run on 8 NeuronCores; returns (out [B,T,C] f32, BassKernelResults)."""
    nc = _get_nc()
    x = np.ascontiguousarray(np.asarray(inputs["x"], dtype=np.float32))
    mask = np.asarray(inputs["mask"])
    if mask.dtype != np.uint8:
        mask = mask.astype(np.uint8)
    mask = np.ascontiguousarray(mask)
    w = np.ascontiguousarray(np.asarray(inputs["W"], dtype=np.float32))
    in_maps = [
        {"x": x[b], "mask": mask[b], "W": w} for b in range(B)
    ]
    res = bass_utils.run_bass_kernel_spmd(
        nc,
        in_maps,
        core_ids=list(range(B)),
        trace=trace,
        tmpdir=tmpdir,
    )
    out = np.stack([res.results[b]["out"] for b in range(B)], axis=0)
    return out, res


def kernel(**inputs) -> np.ndarray:
    out, _ = run(inputs, trace=False)
    return out


# revision 19
# speedup vs baseline: 1.1905x; 1.1905x over previous
"""CrossLinear attention kernel for Trainium2 (8 NeuronCores, data-parallel over batch).

Computes, per batch element b:
    scores = x_b @ x_b^T            [T, T]
    scores[mask] = -inf
    attn = softmax(scores, axis=-1)
    xx = x_b @ W                    [T, C]
    out_b = attn @ xx               [T, C]

with B=8, T=2048, C=1024 (fp32).  One batch element per NeuronCore.

Design notes (v2 — symmetric scores):
  - All big matmuls use float32r operands -> 1 cycle/row on the PE when the
    moving dim is >=256 (plain fp32 is 4 cycles/row).  End-to-end rel err
    stays ~1e-3.
  - scores = x x^T is SYMMETRIC: stripe i only matmuls blocks (i, j>=i);
    blocks (i, j<i) are PE-transposed (1.5 cyc/row, ~5x cheaper than the
    matmul they replace) from raw (pre-mask) 128x128 blocks saved by
    earlier stripes.  Raw blocks are copied PSUM->SBUF on the scalar
    engine before the DVE mask-fill of the same PSUM region; a 64-slot
    interval-colored save pool (exact peak occupancy) holds them.
  - x is transposed once on the PE into xT [C, T] (f32r input via ACT cast
    so the transposes run 1.5 cyc/row); both score-matmul operands and the
    x@W lhsT come from xT.
  - The attn @ xx GEMM runs in bf16: xx is stored bf16 and the attn
    transposes (f32r on the PE) are cast to bf16 on the psum->sbuf copy.
    PSUM accumulates fp32, so only input rounding (~0.2%) is added.
  - Masking via copy_predicated (mask u8, -1e9 fill) per 512-col bank,
    interleaved with the score work; softmax: per-bank DVE row max ->
    combine (negated) -> ACT exp with per-partition bias and accumulated
    row sum -> 1/sum folded into the final output scale.
  - Phase ordering: x DMAs are issued before W so the PE starts early; the
    x-transposes and x@W matmuls interleave; scores(0) is emitted right
    after the last x@W matmul so the PE never drains at the phase
    boundary.  PSUM pools are staged (psA+psXX = 6 banks, then psS joins
    at 4+4 = 8, then psT/psO reuse psXX's banks).
  - This container's walrus accepts at most ONE sync-wait per instruction;
    _split_sync_waits hoists extras onto single-wait NoOps.
"""

import sys

if "/opt/trn_rl_repo" not in sys.path:
    sys.path.insert(0, "/opt/trn_rl_repo")

from contextlib import ExitStack

import numpy as np

import concourse.bass as bass
import concourse.mybir as mybir
import concourse.tile as tile
from concourse import bass_utils
from concourse.bass import ds, ts
from concourse.masks import make_identity

B, T, C = 8, 2048, 1024
P = 128                 # partition block
NT = T // P             # 16 row blocks (stripes)
NKC = C // P            # 8 contraction chunks over C
NKS = T // P            # 16 contraction chunks over T (for attn @ xx)
NSLOTS = 64             # saved raw score blocks (exact interval-coloring peak)
NEG_BIG = -1.0e9

F32 = mybir.dt.float32
F32R = mybir.dt.float32r
BF16 = mybir.dt.bfloat16
U8 = mybir.dt.uint8


def _span_chunks(start_col: int):
    """Bank-aligned (512-col) chunk widths covering [start_col, T)."""
    chunks = []
    col = start_col
    while col < T:
        bank_end = (col // 512 + 1) * 512
        w = min(bank_end, T) - col
        chunks.append((col, w))
        col += w
    return chunks


def build_bass():
    nc = bass.Bass(
        trn_type="TRN2",
        target_bir_lowering=False,
        debug=False,
        enable_asserts=False,
        num_devices=8,
    )
    x_d = nc.dram_tensor("x", [T, C], F32, kind="ExternalInput").ap()
    m_d = nc.dram_tensor("mask", [T, T], U8, kind="ExternalInput").ap()
    w_d = nc.dram_tensor("W", [C, C], F32, kind="ExternalInput").ap()
    o_d = nc.dram_tensor("out", [T, C], F32, kind="ExternalOutput").ap()

    with tile.TileContext(nc) as tc:
        _kernel_body(nc, tc, x_d, m_d, w_d, o_d)
    return nc


def _kernel_body(nc, tc, x_d, m_d, w_d, o_d):
    with ExitStack() as big:
        const = big.enter_context(tc.tile_pool(name="const", bufs=1))
        ident = const.tile([P, P], F32)
        make_identity(nc, ident[:])
        ident_r = const.tile([P, P], F32R)
        nc.vector.tensor_copy(ident_r[:], ident[:])
        negbig = const.tile([P, 512], F32)
        nc.gpsimd.memset(negbig[:], NEG_BIG)

        persist = big.enter_context(tc.tile_pool(name="persist", bufs=1))
        xT = persist.tile([P, NKC, T], F32R)   # xT[p, k, t] = x[t, k*128+p]
        xx = persist.tile([P, NT, C], BF16)    # xx[p, i, d] = (x@W)[i*128+p, d]
        # raw upper-triangle score blocks, producer-contiguous: stripe i's
        # saved span (cols 128(i+1)..T) lives at block offset sbase[i], so
        # one batched ACT copy per 512-col bank saves it.  block (j,i)
        # (consumed by stripe j>i) sits at sbase[i] + (j-i-1).
        sbase = [0]
        for i in range(1, NT):
            sbase.append(sbase[-1] + (NT - 1 - i + 1))
        save = persist.tile([P, sbase[-1] * P], F32R)

        def save_cols(j, i):
            """column range in `save` holding raw block (j, i), j > i."""
            return ds((sbase[i] + (j - i - 1)) * P, P)

        # W (f32r, 32KB/partition) is dead once the last x@W matmul ran and
        # `save` is first written in phase B -- alias W onto save's first
        # 8K columns; the framework's WAR tracking orders the reuse.
        def wv(k, h):
            return save[:, ds(k * C + h * 512, 512)]

        # ---- Phase A: transpose x into xT; compute xx = x @ W ----
        # f32r is raw fp32 bytes (the TF32-style rounding happens inside the
        # PE), so x and W DMA straight into f32r tiles via bitcast views --
        # no cast instructions, and the PE can start as soon as x[0] lands.
        phA = ExitStack()
        xload = phA.enter_context(tc.tile_pool(name="xload", bufs=3))
        psXX = phA.enter_context(tc.tile_pool(name="psXX", bufs=2, space="PSUM"))
        psA = phA.enter_context(tc.tile_pool(name="psA", bufs=2, space="PSUM"))

        xts = {}

        def xdma(i):
            t = xload.tile([P, C], F32R, tag="xt")
            nc.sync.dma_start(t[:], x_d[ts(i, P), :].bitcast(F32R))
            xts[i] = t

        def xpose_c(i):
            xr = xts.pop(i)
            for g in range(2):
                pt = psA.tile([P, 4, P], F32R, tag="pt")
                for j in range(4):
                    k = g * 4 + j
                    nc.tensor.transpose(
                        pt[:, j, :], xr[:, ds(k * P, P)], ident_r[:]
                    )
                nc.vector.tensor_copy(
                    xT[:, ds(g * 4, 4), ds(i * P, P)], pt[:]
                )

        def xxmm(i):
            po = psXX.tile([P, C], F32, tag="po1")
            for k in range(NKC):
                for h in range(2):
                    nc.tensor.matmul(
                        po[:, ds(h * 512, 512)],
                        lhsT=xT[:, k, ds(i * P, P)],
                        rhs=wv(k, h),
                        start=(k == 0),
                        stop=(k == NKC - 1),
                    )
            nc.vector.tensor_copy(xx[:, i, :], po[:])

        # x DMAs first so the PE can start early; W loads overlap.
        xdma(0)
        xdma(1)
        xdma(2)
        for k in range(NKC):
            nc.sync.dma_start(
                save[:, ds(k * C, C)], w_d[ts(k, P), :].bitcast(F32R)
            )

        xpose_c(0)
        for i in range(1, NT):
            xpose_c(i)
            if i + 2 < NT:
                xdma(i + 2)
            xxmm(i - 1)
        xxmm(NT - 1)

        # release phase-A SBUF + PSUM so phase B can reuse the space; the
        # PE queue still flows straight from xx(15) into scores(0)
        phA.close()

        # ---- Phase B: attention main loop over row stripes ----
        with tc.tile_pool(name="maskp", bufs=2) as maskp, \
             tc.tile_pool(name="ppool", bufs=2) as ppool, \
             tc.tile_pool(name="ptpool", bufs=2) as ptpool, \
             tc.tile_pool(name="opool", bufs=2) as opool, \
             tc.tile_pool(name="stats", bufs=4) as stats, \
             tc.tile_pool(name="psS", bufs=1, space="PSUM") as psS, \
             tc.tile_pool(name="psT", bufs=2, space="PSUM") as psT, \
             tc.tile_pool(name="psO", bufs=1, space="PSUM") as psO:

            def scores(i):
                """Score stripe i, emitted bank-by-bank (512 cols) so the DVE
                mask/rowmax chain and the ACT raw-block save of bank n overlap
                the PE work of banks n+1..: per bank emit [transposes of saved
                raw blocks (j<i)] [matmul chunk (j>=i)] [batched raw save]
                [mask-fill] [row max]."""
                msk = maskp.tile([P, T], U8, tag="mask")
                nc.sync.dma_start(msk[:], m_d[ts(i, P), :])
                ps = psS.tile([P, T], F32, tag="scores")
                maxpart = stats.tile([P, 4], F32, tag="maxpart")
                for n in range(4):
                    sl = ds(n * 512, 512)
                    b0, b1 = n * 4, n * 4 + 4   # 128-col blocks of this bank
                    # left part: PE-transpose saved raw blocks
                    for j in range(b0, min(b1, i)):
                        nc.tensor.transpose(
                            ps[:, ds(j * P, P)].bitcast(F32R),
                            save[:, save_cols(i, j)],
                            ident_r[:],
                        )
                    # right part: one bank-aligned matmul chunk
                    mm0 = max(b0 * P, i * P)
                    w = (b1 * P) - mm0
                    if w > 0:
                        msl = ds(mm0, w)
                        for k in range(NKC):
                            nc.tensor.matmul(
                                ps[:, msl],
                                lhsT=xT[:, k, ds(i * P, P)],
                                rhs=xT[:, k, msl],
                                start=(k == 0),
                                stop=(k == NKC - 1),
                            )
                    # batched raw save of this bank's j>i blocks (pre-mask)
                    s0 = max(b0, i + 1)
                    if s0 < b1:
                        nw = (b1 - s0) * P
                        nc.scalar.copy(
                            save[:, ds((sbase[i] + s0 - i - 1) * P, nw)],
                            ps[:, ds(s0 * P, nw)],
                        )
                    # mask + row max (DVE)
                    nc.vector.copy_predicated(ps[:, sl], msk[:, sl], negbig[:])
                    nc.vector.reduce_max(
                        maxpart[:, ds(n, 1)], ps[:, sl],
                        axis=mybir.AxisListType.X,
                    )
                return ps, maxpart

            def softmax(i, ps, maxpart):
                negmax = stats.tile([P, 1], F32, tag="negmax")
                nc.vector.reduce_max(
                    negmax[:], maxpart[:], axis=mybir.AxisListType.X, negate=True
                )
                # exp in two halves: pv_transpose groups 0-1 and the next
                # stripe's left-half PSUM writes only wait on exp half 0,
                # shaving ~1us off the per-stripe critical path
                rowsums = stats.tile([P, 2], F32, tag="rowsums")
                p_i = ppool.tile([P, T], F32R, tag="p")
                for g in range(2):
                    sl = ds(g * 1024, 1024)
                    nc.scalar.activation(
                        p_i[:, sl],
                        ps[:, sl],
                        mybir.ActivationFunctionType.Exp,
                        bias=negmax[:],
                        scale=1.0,
                        accum_out=rowsums[:, ds(g, 1)],
                    )
                rowsum = stats.tile([P, 1], F32, tag="rowsum")
                nc.vector.reduce_sum(
                    rowsum[:], rowsums[:], axis=mybir.AxisListType.X
                )
                recip = stats.tile([P, 1], F32, tag="recip")
                nc.vector.reciprocal(recip[:], rowsum[:])
                return p_i, recip

            def pv_transpose(i, p_i):
                """PE-transpose attn weights (f32r); psum->sbuf casts to
                bf16 on the scalar engine."""
                pT = ptpool.tile([P, NKS, P], BF16, tag="pT")
                for g in range(4):
                    pt_ps = psT.tile([P, 4, P], F32R, tag="ptps")
                    for j in range(4):
                        s = g * 4 + j
                        nc.tensor.transpose(
                            pt_ps[:, j, :], p_i[:, ds(s * P, P)], ident_r[:]
                        )
                    nc.scalar.copy(pT[:, ds(g * 4, 4), :], pt_ps[:])
                return pT

            def pv_out(i, pT, recip):
                po = psO.tile([P, C], F32, tag="po2")
                for s in range(NKS):
                    for h in range(2):
                        nc.tensor.matmul(
                            po[:, ds(h * 512, 512)],
                            lhsT=pT[:, s, :],
                            rhs=xx[:, s, ds(h * 512, 512)],
                            start=(s == 0),
                            stop=(s == NKS - 1),
                        )
                out_t = opool.tile([P, C], F32, tag="out")
                # on the DVE: ACT must stay clear so exp(i+1) starts the
                # moment negmax lands (it gates all next-stripe PE work)
                nc.vector.tensor_scalar_mul(out_t[:], po[:], recip[:])
                nc.sync.dma_start(o_d[ts(i, P), :], out_t[:])

            # software pipeline; engine-queue order is the point
            sc = scores(0)
            sm = softmax(0, *sc)
            for i in range(NT):
                nxt_sc = scores(i + 1) if i + 1 < NT else None
                pT = pv_transpose(i, sm[0])
                nxt_sm = softmax(i + 1, *nxt_sc) if nxt_sc else None
                pv_out(i, pT, sm[1])
                sm = nxt_sm


def _split_sync_waits(nc, limit: int = 1):
    """The walrus build in this container rejects instructions with more than
    one sync-wait command.  Hoist excess waits onto preceding single-wait
    NoOps on the same engine (waits execute in order before the original
    instruction, so semantics are preserved)."""
    n_new = 0
    for fn in nc.m.functions:
        for blk in fn.blocks:
            new_insts = []
            for inst in blk.instructions:
                si = inst.sync_info
                if si and si.on_wait and len(si.on_wait) > limit:
                    waits = list(si.on_wait)
                    extra, keep = waits[:-limit], waits[-limit:]
                    for w in extra:
                        nop = mybir.InstNoOp(
                            name=f"{inst.name}-wsplit{n_new}", ins=[], outs=[]
                        )
                        n_new += 1
                        nop.engine = inst.engine
                        nop.sync_info = mybir.SyncInfo(on_wait=[w], on_update=[])
                        new_insts.append(nop)
                    si.on_wait[:] = keep
                new_insts.append(inst)
            blk.instructions[:] = new_insts
    return n_new


_NC_CACHE = None


def _get_nc():
    global _NC_CACHE
    if _NC_CACHE is None:
        nc = build_bass()
        _split_sync_waits(nc, limit=1)
        _NC_CACHE = nc
    return _NC_CACHE


def run(inputs: dict, trace: bool = False, tmpdir: str | None = None):
    """Run on 8 NeuronCores; returns (out [B,T,C] f32, BassKernelResults)."""
    nc = _get_nc()
    x = np.ascontiguousarray(np.asarray(inputs["x"], dtype=np.float32))
    mask = np.asarray(inputs["mask"])
    if mask.dtype != np.uint8:
        mask = mask.astype(np.uint8)
    mask = np.ascontiguousarray(mask)
    w = np.ascontiguousarray(np.asarray(inputs["W"], dtype=np.float32))
    in_maps = [
        {"x": x[b], "mask": mask[b], "W": w} for b in range(B)
    ]
    res = bass_utils.run_bass_kernel_spmd(
        nc,
        in_maps,
        core_ids=list(range(B)),
        trace=trace,
        tmpdir=tmpdir,
    )
    out = np.stack([res.results[b]["out"] for b in range(B)], axis=0)
    return out, res


def kernel(**inputs) -> np.ndarray:
    out, _ = run(inputs, trace=False)
    return out


# revision 20
# speedup vs baseline: 1.2349x; 1.0373x over previous
"""CrossLinear attention kernel for Trainium2 (8 NeuronCores, data-parallel over batch).

Computes, per batch element b:
    scores = x_b @ x_b^T            [T, T]
    scores[mask] = -inf
    attn = softmax(scores, axis=-1)
    xx = x_b @ W                    [T, C]
    out_b = attn @ xx               [T, C]

with B=8, T=2048, C=1024 (fp32).  One batch element per NeuronCore.

Design notes (v2 — symmetric scores):
  - All big matmuls use float32r operands -> 1 cycle/row on the PE when the
    moving dim is >=256 (plain fp32 is 4 cycles/row).  End-to-end rel err
    stays ~1e-3.
  - scores = x x^T is SYMMETRIC: stripe i only matmuls blocks (i, j>=i);
    blocks (i, j<i) are PE-transposed (1.5 cyc/row, ~5x cheaper than the
    matmul they replace) from raw (pre-mask) 128x128 blocks saved by
    earlier stripes.  Raw blocks are copied PSUM->SBUF on the scalar
    engine before the DVE mask-fill of the same PSUM region; a 64-slot
    interval-colored save pool (exact peak occupancy) holds them.
  - x is transposed once on the PE into xT [C, T] (f32r input via ACT cast
    so the transposes run 1.5 cyc/row); both score-matmul operands and the
    x@W lhsT come from xT.
  - The attn @ xx GEMM runs in bf16: xx is stored bf16 and the attn
    transposes (f32r on the PE) are cast to bf16 on the psum->sbuf copy.
    PSUM accumulates fp32, so only input rounding (~0.2%) is added.
  - Masking via copy_predicated (mask u8, -1e9 fill) per 512-col bank,
    interleaved with the score work; softmax: per-bank DVE row max ->
    combine (negated) -> ACT exp with per-partition bias and accumulated
    row sum -> 1/sum folded into the final output scale.
  - Phase ordering: x DMAs are issued before W so the PE starts early; the
    x-transposes and x@W matmuls interleave; scores(0) is emitted right
    after the last x@W matmul so the PE never drains at the phase
    boundary.  PSUM pools are staged (psA+psXX = 6 banks, then psS joins
    at 4+4 = 8, then psT/psO reuse psXX's banks).
  - This container's walrus accepts at most ONE sync-wait per instruction;
    _split_sync_waits hoists extras onto single-wait NoOps.
"""

import sys

if "/opt/trn_rl_repo" not in sys.path:
    sys.path.insert(0, "/opt/trn_rl_repo")

from contextlib import ExitStack

import numpy as np

import concourse.bass as bass
import concourse.mybir as mybir
import concourse.tile as tile
from concourse import bass_utils
from concourse.bass import ds, ts
from concourse.masks import make_identity

B, T, C = 8, 2048, 1024
P = 128                 # partition block
NT = T // P             # 16 row blocks (stripes)
NKC = C // P            # 8 contraction chunks over C
NKS = T // P            # 16 contraction chunks over T (for attn @ xx)
NSLOTS = 64             # saved raw score blocks (exact interval-coloring peak)
NEG_BIG = -1.0e9

F32 = mybir.dt.float32
F32R = mybir.dt.float32r
BF16 = mybir.dt.bfloat16
U8 = mybir.dt.uint8


def _span_chunks(start_col: int):
    """Bank-aligned (512-col) chunk widths covering [start_col, T)."""
    chunks = []
    col = start_col
    while col < T:
        bank_end = (col // 512 + 1) * 512
        w = min(bank_end, T) - col
        chunks.append((col, w))
        col += w
    return chunks


def build_bass():
    nc = bass.Bass(
        trn_type="TRN2",
        target_bir_lowering=False,
        debug=False,
        enable_asserts=False,
        num_devices=8,
    )
    x_d = nc.dram_tensor("x", [T, C], F32, kind="ExternalInput").ap()
    m_d = nc.dram_tensor("mask", [T, T], U8, kind="ExternalInput").ap()
    w_d = nc.dram_tensor("W", [C, C], F32, kind="ExternalInput").ap()
    o_d = nc.dram_tensor("out", [T, C], F32, kind="ExternalOutput").ap()

    with tile.TileContext(nc) as tc:
        _kernel_body(nc, tc, x_d, m_d, w_d, o_d)
    return nc


def _kernel_body(nc, tc, x_d, m_d, w_d, o_d):
    with ExitStack() as big:
        const = big.enter_context(tc.tile_pool(name="const", bufs=1))
        ident = const.tile([P, P], F32)
        make_identity(nc, ident[:])
        ident_r = const.tile([P, P], F32R)
        nc.vector.tensor_copy(ident_r[:], ident[:])
        negbig = const.tile([P, 512], F32)
        nc.gpsimd.memset(negbig[:], NEG_BIG)

        persist = big.enter_context(tc.tile_pool(name="persist", bufs=1))
        xT = persist.tile([P, NKC, T], F32R)   # xT[p, k, t] = x[t, k*128+p]
        xx = persist.tile([P, NT, C], BF16)    # xx[p, i, d] = (x@W)[i*128+p, d]
        # raw upper-triangle score blocks, producer-contiguous: stripe i's
        # saved span (cols 128(i+1)..T) lives at block offset sbase[i], so
        # one batched ACT copy per 512-col bank saves it.  block (j,i)
        # (consumed by stripe j>i) sits at sbase[i] + (j-i-1).
        sbase = [0]
        for i in range(1, NT):
            sbase.append(sbase[-1] + (NT - 1 - i + 1))
        save = persist.tile([P, sbase[-1] * P], F32R)

        def save_cols(j, i):
            """column range in `save` holding raw block (j, i), j > i."""
            return ds((sbase[i] + (j - i - 1)) * P, P)

        # W (f32r, 32KB/partition) is dead once the last x@W matmul ran and
        # `save` is first written in phase B -- alias W onto save's first
        # 8K columns; the framework's WAR tracking orders the reuse.
        def wv(k, h):
            return save[:, ds(k * C + h * 512, 512)]

        # ---- Phase A: transpose x into xT; compute xx = x @ W ----
        # f32r is raw fp32 bytes (the TF32-style rounding happens inside the
        # PE), so x and W DMA straight into f32r tiles via bitcast views --
        # no cast instructions, and the PE can start as soon as x[0] lands.
        phA = ExitStack()
        xload = phA.enter_context(tc.tile_pool(name="xload", bufs=3))
        psXX = phA.enter_context(tc.tile_pool(name="psXX", bufs=2, space="PSUM"))
        psA = phA.enter_context(tc.tile_pool(name="psA", bufs=2, space="PSUM"))

        xts = {}

        def xdma(i):
            t = xload.tile([P, C], F32R, tag="xt")
            nc.sync.dma_start(t[:], x_d[ts(i, P), :].bitcast(F32R))
            xts[i] = t

        def xpose_c(i):
            xr = xts.pop(i)
            for g in range(2):
                pt = psA.tile([P, 4, P], F32R, tag="pt")
                for j in range(4):
                    k = g * 4 + j
                    nc.tensor.transpose(
                        pt[:, j, :], xr[:, ds(k * P, P)], ident_r[:]
                    )
                nc.vector.tensor_copy(
                    xT[:, ds(g * 4, 4), ds(i * P, P)], pt[:]
                )

        def xxmm(i):
            po = psXX.tile([P, C], F32, tag="po1")
            for k in range(NKC):
                for h in range(2):
                    nc.tensor.matmul(
                        po[:, ds(h * 512, 512)],
                        lhsT=xT[:, k, ds(i * P, P)],
                        rhs=wv(k, h),
                        start=(k == 0),
                        stop=(k == NKC - 1),
                    )
            nc.vector.tensor_copy(xx[:, i, :], po[:])

        # x DMAs first so the PE can start early; W loads overlap.
        xdma(0)
        xdma(1)
        xdma(2)
        for k in range(NKC):
            nc.sync.dma_start(
                save[:, ds(k * C, C)], w_d[ts(k, P), :].bitcast(F32R)
            )

        xpose_c(0)
        for i in range(1, NT):
            xpose_c(i)
            if i + 2 < NT:
                xdma(i + 2)
            xxmm(i - 1)
        xxmm(NT - 1)

        # release phase-A SBUF + PSUM so phase B can reuse the space; the
        # PE queue still flows straight from xx(15) into scores(0)
        phA.close()

        # ---- Phase B: attention main loop over row stripes ----
        with tc.tile_pool(name="maskp", bufs=2) as maskp, \
             tc.tile_pool(name="ppool", bufs=2) as ppool, \
             tc.tile_pool(name="ptpool", bufs=2) as ptpool, \
             tc.tile_pool(name="opool", bufs=2) as opool, \
             tc.tile_pool(name="stats", bufs=4) as stats, \
             tc.tile_pool(name="psS", bufs=1, space="PSUM") as psS, \
             tc.tile_pool(name="psT", bufs=2, space="PSUM") as psT, \
             tc.tile_pool(name="psO", bufs=1, space="PSUM") as psO:

            def scores(i):
                """Score stripe i, emitted bank-by-bank (512 cols) so the DVE
                mask/rowmax chain and the ACT raw-block save of bank n overlap
                the PE work of banks n+1..: per bank emit [transposes of saved
                raw blocks (j<i)] [matmul chunk (j>=i)] [batched raw save]
                [mask-fill] [row max]."""
                msk = maskp.tile([P, T], U8, tag="mask")
                nc.sync.dma_start(msk[:], m_d[ts(i, P), :])
                ps = psS.tile([P, T], F32, tag="scores")
                maxpart = stats.tile([P, 4], F32, tag="maxpart")
                for n in range(4):
                    sl = ds(n * 512, 512)
                    b0, b1 = n * 4, n * 4 + 4   # 128-col blocks of this bank
                    # left part: PE-transpose saved raw blocks
                    for j in range(b0, min(b1, i)):
                        nc.tensor.transpose(
                            ps[:, ds(j * P, P)].bitcast(F32R),
                            save[:, save_cols(i, j)],
                            ident_r[:],
                        )
                    # right part: one bank-aligned matmul chunk
                    mm0 = max(b0 * P, i * P)
                    w = (b1 * P) - mm0
                    if w > 0:
                        msl = ds(mm0, w)
                        for k in range(NKC):
                            nc.tensor.matmul(
                                ps[:, msl],
                                lhsT=xT[:, k, ds(i * P, P)],
                                rhs=xT[:, k, msl],
                                start=(k == 0),
                                stop=(k == NKC - 1),
                            )
                    # batched raw save of this bank's j>i blocks (pre-mask)
                    s0 = max(b0, i + 1)
                    if s0 < b1:
                        nw = (b1 - s0) * P
                        nc.scalar.copy(
                            save[:, ds((sbase[i] + s0 - i - 1) * P, nw)],
                            ps[:, ds(s0 * P, nw)],
                        )
                    # mask + row max (DVE)
                    nc.vector.copy_predicated(ps[:, sl], msk[:, sl], negbig[:])
                    nc.vector.reduce_max(
                        maxpart[:, ds(n, 1)], ps[:, sl],
                        axis=mybir.AxisListType.X,
                    )
                return ps, maxpart

            def softmax(i, ps, maxpart):
                negmax = stats.tile([P, 1], F32, tag="negmax")
                nc.vector.reduce_max(
                    negmax[:], maxpart[:], axis=mybir.AxisListType.X, negate=True
                )
                rowsum = stats.tile([P, 1], F32, tag="rowsum")
                p_i = ppool.tile([P, T], F32R, tag="p")
                nc.scalar.activation(
                    p_i[:],
                    ps[:],
                    mybir.ActivationFunctionType.Exp,
                    bias=negmax[:],
                    scale=1.0,
                    accum_out=rowsum[:],
                )
                recip = stats.tile([P, 1], F32, tag="recip")
                nc.vector.reciprocal(recip[:], rowsum[:])
                return p_i, recip

            def pv_transpose(i, p_i):
                """PE-transpose attn weights (f32r); psum->sbuf casts to
                bf16 on the scalar engine."""
                pT = ptpool.tile([P, NKS, P], BF16, tag="pT")
                for g in range(4):
                    pt_ps = psT.tile([P, 4, P], F32R, tag="ptps")
                    for j in range(4):
                        s = g * 4 + j
                        nc.tensor.transpose(
                            pt_ps[:, j, :], p_i[:, ds(s * P, P)], ident_r[:]
                        )
                    nc.scalar.copy(pT[:, ds(g * 4, 4), :], pt_ps[:])
                return pT

            def pv_out(i, pT, recip):
                po = psO.tile([P, C], F32, tag="po2")
                for s in range(NKS):
                    for h in range(2):
                        nc.tensor.matmul(
                            po[:, ds(h * 512, 512)],
                            lhsT=pT[:, s, :],
                            rhs=xx[:, s, ds(h * 512, 512)],
                            start=(s == 0),
                            stop=(s == NKS - 1),
                        )
                out_t = opool.tile([P, C], F32, tag="out")
                # on the DVE: ACT must stay clear so exp(i+1) starts the
                # moment negmax lands (it gates all next-stripe PE work)
                nc.vector.tensor_scalar_mul(out_t[:], po[:], recip[:])
                nc.sync.dma_start(o_d[ts(i, P), :], out_t[:])

            # software pipeline; engine-queue order is the point
            sc = scores(0)
            sm = softmax(0, *sc)
            for i in range(NT):
                nxt_sc = scores(i + 1) if i + 1 < NT else None
                pT = pv_transpose(i, sm[0])
                nxt_sm = softmax(i + 1, *nxt_sc) if nxt_sc else None
                pv_out(i, pT, sm[1])
                sm = nxt_sm


def _split_sync_waits(nc, limit: int = 1):
    """The walrus build in this container rejects instructions with more than
    one sync-wait command.  Hoist excess waits onto preceding single-wait
    NoOps on the same engine (waits execute in order before the original
    instruction, so semantics are preserved)."""
    n_new = 0
    for fn in nc.m.functions:
        for blk in fn.blocks:
            new_insts = []
            for inst in blk.instructions:
                si = inst.sync_info
                if si and si.on_wait and len(si.on_wait) > limit:
                    waits = list(si.on_wait)
                    extra, keep = waits[:-limit], waits[-limit:]
                    for w in extra:
                        nop = mybir.InstNoOp(
                            name=f"{inst.name}-wsplit{n_new}", ins=[], outs=[]
                        )
                        n_new += 1
                        nop.engine = inst.engine
                        nop.sync_info = mybir.SyncInfo(on_wait=[w], on_update=[])
                        new_insts.append(nop)
                    si.on_wait[:] = keep
                new_insts.append(inst)
            blk.instructions[:] = new_insts
    return n_new


_NC_CACHE = None


def _get_nc():
    global _NC_CACHE
    if _NC_CACHE is None:
        nc = build_bass()
        _split_sync_waits(nc, limit=1)
        _NC_CACHE = nc
    return _NC_CACHE


def run(inputs: dict, trace: bool = False, tmpdir: str | None = None):
    """Run on 8 NeuronCores; returns (out [B,T,C] f32, BassKernelResults)."""
    nc = _get_nc()
    x = np.ascontiguousarray(np.asarray(inputs["x"], dtype=np.float32))
    mask = np.asarray(inputs["mask"])
    if mask.dtype != np.uint8:
        mask = mask.astype(np.uint8)
    mask = np.ascontiguousarray(mask)
    w = np.ascontiguousarray(np.asarray(inputs["W"], dtype=np.float32))
    in_maps = [
        {"x": x[b], "mask": mask[b], "W": w} for b in range(B)
    ]
    res = bass_utils.run_bass_kernel_spmd(
        nc,
        in_maps,
        core_ids=list(range(B)),
        trace=trace,
        tmpdir=tmpdir,
    )
    out = np.stack([res.results[b]["out"] for b in range(B)], axis=0)
    return out, res


def kernel(**inputs) -> np.ndarray:
    out, _ = run(inputs, trace=False)
    return out


# revision 21
# speedup vs baseline: 1.2389x; 1.0033x over previous
"""CrossLinear attention kernel for Trainium2 (8 NeuronCores, data-parallel over batch).

Computes, per batch element b:
    scores = x_b @ x_b^T            [T, T]
    scores[mask] = -inf
    attn = softmax(scores, axis=-1)
    xx = x_b @ W                    [T, C]
    out_b = attn @ xx               [T, C]

with B=8, T=2048, C=1024 (fp32).  One batch element per NeuronCore.

Design notes (v2 — symmetric scores):
  - All big matmuls use float32r operands -> 1 cycle/row on the PE when the
    moving dim is >=256 (plain fp32 is 4 cycles/row).  End-to-end rel err
    stays ~1e-3.
  - scores = x x^T is SYMMETRIC: stripe i only matmuls blocks (i, j>=i);
    blocks (i, j<i) are PE-transposed (1.5 cyc/row, ~5x cheaper than the
    matmul they replace) from raw (pre-mask) 128x128 blocks saved by
    earlier stripes.  Raw blocks are copied PSUM->SBUF on the scalar
    engine before the DVE mask-fill of the same PSUM region; a 64-slot
    interval-colored save pool (exact peak occupancy) holds them.
  - x is transposed once on the PE into xT [C, T] (f32r input via ACT cast
    so the transposes run 1.5 cyc/row); both score-matmul operands and the
    x@W lhsT come from xT.
  - The attn @ xx GEMM runs in bf16: xx is stored bf16 and the attn
    transposes (f32r on the PE) are cast to bf16 on the psum->sbuf copy.
    PSUM accumulates fp32, so only input rounding (~0.2%) is added.
  - Masking via copy_predicated (mask u8, -1e9 fill) per 512-col bank,
    interleaved with the score work; softmax: per-bank DVE row max ->
    combine (negated) -> ACT exp with per-partition bias and accumulated
    row sum -> 1/sum folded into the final output scale.
  - Phase ordering: x DMAs are issued before W so the PE starts early; the
    x-transposes and x@W matmuls interleave; scores(0) is emitted right
    after the last x@W matmul so the PE never drains at the phase
    boundary.  PSUM pools are staged (psA+psXX = 6 banks, then psS joins
    at 4+4 = 8, then psT/psO reuse psXX's banks).
  - This container's walrus accepts at most ONE sync-wait per instruction;
    _split_sync_waits hoists extras onto single-wait NoOps.
"""

import sys

if "/opt/trn_rl_repo" not in sys.path:
    sys.path.insert(0, "/opt/trn_rl_repo")

from contextlib import ExitStack

import numpy as np

import concourse.bass as bass
import concourse.mybir as mybir
import concourse.tile as tile
from concourse import bass_utils
from concourse.bass import ds, ts
from concourse.masks import make_identity

B, T, C = 8, 2048, 1024
P = 128                 # partition block
NT = T // P             # 16 row blocks (stripes)
NKC = C // P            # 8 contraction chunks over C
NKS = T // P            # 16 contraction chunks over T (for attn @ xx)
NSLOTS = 64             # saved raw score blocks (exact interval-coloring peak)
NEG_BIG = -1.0e9

F32 = mybir.dt.float32
F32R = mybir.dt.float32r
BF16 = mybir.dt.bfloat16
U8 = mybir.dt.uint8


def _span_chunks(start_col: int):
    """Bank-aligned (512-col) chunk widths covering [start_col, T)."""
    chunks = []
    col = start_col
    while col < T:
        bank_end = (col // 512 + 1) * 512
        w = min(bank_end, T) - col
        chunks.append((col, w))
        col += w
    return chunks


def build_bass():
    nc = bass.Bass(
        trn_type="TRN2",
        target_bir_lowering=False,
        debug=False,
        enable_asserts=False,
        num_devices=8,
    )
    x_d = nc.dram_tensor("x", [T, C], F32, kind="ExternalInput").ap()
    m_d = nc.dram_tensor("mask", [T, T], U8, kind="ExternalInput").ap()
    w_d = nc.dram_tensor("W", [C, C], F32, kind="ExternalInput").ap()
    o_d = nc.dram_tensor("out", [T, C], F32, kind="ExternalOutput").ap()

    with tile.TileContext(nc) as tc:
        _kernel_body(nc, tc, x_d, m_d, w_d, o_d)
    return nc


def _kernel_body(nc, tc, x_d, m_d, w_d, o_d):
    with ExitStack() as big:
        const = big.enter_context(tc.tile_pool(name="const", bufs=1))
        ident = const.tile([P, P], F32)
        make_identity(nc, ident[:])
        ident_r = const.tile([P, P], F32R)
        nc.vector.tensor_copy(ident_r[:], ident[:])
        negbig = const.tile([P, 512], F32)
        nc.gpsimd.memset(negbig[:], NEG_BIG)

        persist = big.enter_context(tc.tile_pool(name="persist", bufs=1))
        xT = persist.tile([P, NKC, T], F32R)   # xT[p, k, t] = x[t, k*128+p]
        xx = persist.tile([P, NT, C], BF16)    # xx[p, i, d] = (x@W)[i*128+p, d]
        # raw upper-triangle score blocks, producer-contiguous: stripe i's
        # saved span (cols 128(i+1)..T) lives at block offset sbase[i], so
        # one batched ACT copy per 512-col bank saves it.  block (j,i)
        # (consumed by stripe j>i) sits at sbase[i] + (j-i-1).
        sbase = [0]
        for i in range(1, NT):
            sbase.append(sbase[-1] + (NT - 1 - i + 1))
        save = persist.tile([P, sbase[-1] * P], F32R)

        def save_cols(j, i):
            """column range in `save` holding raw block (j, i), j > i."""
            return ds((sbase[i] + (j - i - 1)) * P, P)

        # W (f32r, 32KB/partition) is dead once the last x@W matmul ran and
        # `save` is first written in phase B -- alias W onto save's first
        # 8K columns; the framework's WAR tracking orders the reuse.
        def wv(k, h):
            return save[:, ds(k * C + h * 512, 512)]

        # ---- Phase A: transpose x into xT; compute xx = x @ W ----
        # f32r is raw fp32 bytes (the TF32-style rounding happens inside the
        # PE), so x and W DMA straight into f32r tiles via bitcast views --
        # no cast instructions, and the PE can start as soon as x[0] lands.
        phA = ExitStack()
        xload = phA.enter_context(tc.tile_pool(name="xload", bufs=3))
        psXX = phA.enter_context(tc.tile_pool(name="psXX", bufs=2, space="PSUM"))
        psA = phA.enter_context(tc.tile_pool(name="psA", bufs=2, space="PSUM"))

        xts = {}

        def xdma(i):
            t = xload.tile([P, C], F32R, tag="xt")
            nc.sync.dma_start(t[:], x_d[ts(i, P), :].bitcast(F32R))
            xts[i] = t

        def xpose_c(i):
            xr = xts.pop(i)
            for g in range(2):
                pt = psA.tile([P, 4, P], F32R, tag="pt")
                for j in range(4):
                    k = g * 4 + j
                    nc.tensor.transpose(
                        pt[:, j, :], xr[:, ds(k * P, P)], ident_r[:]
                    )
                nc.vector.tensor_copy(
                    xT[:, ds(g * 4, 4), ds(i * P, P)], pt[:]
                )

        def xxmm(i):
            po = psXX.tile([P, C], F32, tag="po1")
            for k in range(NKC):
                for h in range(2):
                    nc.tensor.matmul(
                        po[:, ds(h * 512, 512)],
                        lhsT=xT[:, k, ds(i * P, P)],
                        rhs=wv(k, h),
                        start=(k == 0),
                        stop=(k == NKC - 1),
                    )
            nc.vector.tensor_copy(xx[:, i, :], po[:])

        # x DMAs first so the PE can start early; W loads overlap.
        xdma(0)
        xdma(1)
        xdma(2)
        for k in range(NKC):
            nc.sync.dma_start(
                save[:, ds(k * C, C)], w_d[ts(k, P), :].bitcast(F32R)
            )

        xpose_c(0)
        for i in range(1, NT):
            xpose_c(i)
            if i + 2 < NT:
                xdma(i + 2)
            xxmm(i - 1)
        xxmm(NT - 1)

        # release phase-A SBUF + PSUM so phase B can reuse the space; the
        # PE queue still flows straight from xx(15) into scores(0)
        phA.close()

        # ---- Phase B: attention main loop over row stripes ----
        with tc.tile_pool(name="maskp", bufs=2) as maskp, \
             tc.tile_pool(name="ppool", bufs=2) as ppool, \
             tc.tile_pool(name="ptpool", bufs=2) as ptpool, \
             tc.tile_pool(name="opool", bufs=2) as opool, \
             tc.tile_pool(name="stats", bufs=4) as stats, \
             tc.tile_pool(name="psS", bufs=1, space="PSUM") as psS, \
             tc.tile_pool(name="psT", bufs=2, space="PSUM") as psT, \
             tc.tile_pool(name="psO", bufs=1, space="PSUM") as psO:

            def scores(i):
                """Score stripe i, emitted bank-by-bank (512 cols) so the DVE
                mask/rowmax chain and the ACT raw-block save of bank n overlap
                the PE work of banks n+1..: per bank emit [transposes of saved
                raw blocks (j<i)] [matmul chunk (j>=i)] [batched raw save]
                [mask-fill] [row max]."""
                msk = maskp.tile([P, T], U8, tag="mask")
                nc.sync.dma_start(msk[:], m_d[ts(i, P), :])
                ps = psS.tile([P, T], F32, tag="scores")
                maxpart = stats.tile([P, 4], F32, tag="maxpart")
                for n in range(4):
                    sl = ds(n * 512, 512)
                    b0, b1 = n * 4, n * 4 + 4   # 128-col blocks of this bank
                    # left part: PE-transpose saved raw blocks
                    for j in range(b0, min(b1, i)):
                        nc.tensor.transpose(
                            ps[:, ds(j * P, P)].bitcast(F32R),
                            save[:, save_cols(i, j)],
                            ident_r[:],
                        )
                    # right part: one bank-aligned matmul chunk
                    mm0 = max(b0 * P, i * P)
                    w = (b1 * P) - mm0
                    if w > 0:
                        msl = ds(mm0, w)
                        for k in range(NKC):
                            nc.tensor.matmul(
                                ps[:, msl],
                                lhsT=xT[:, k, ds(i * P, P)],
                                rhs=xT[:, k, msl],
                                start=(k == 0),
                                stop=(k == NKC - 1),
                            )
                    # batched raw save of this bank's j>i blocks (pre-mask)
                    s0 = max(b0, i + 1)
                    if s0 < b1:
                        nw = (b1 - s0) * P
                        nc.scalar.copy(
                            save[:, ds((sbase[i] + s0 - i - 1) * P, nw)],
                            ps[:, ds(s0 * P, nw)],
                        )
                    # mask + row max (DVE)
                    nc.vector.copy_predicated(ps[:, sl], msk[:, sl], negbig[:])
                    nc.vector.reduce_max(
                        maxpart[:, ds(n, 1)], ps[:, sl],
                        axis=mybir.AxisListType.X,
                    )
                return ps, maxpart

            def softmax(i, ps, maxpart):
                negmax = stats.tile([P, 1], F32, tag="negmax")
                nc.vector.reduce_max(
                    negmax[:], maxpart[:], axis=mybir.AxisListType.X, negate=True
                )
                rowsum = stats.tile([P, 1], F32, tag="rowsum")
                p_i = ppool.tile([P, T], F32R, tag="p")
                nc.scalar.activation(
                    p_i[:],
                    ps[:],
                    mybir.ActivationFunctionType.Exp,
                    bias=negmax[:],
                    scale=1.0,
                    accum_out=rowsum[:],
                )
                recip = stats.tile([P, 1], F32, tag="recip")
                nc.vector.reciprocal(recip[:], rowsum[:])
                return p_i, recip

            def pv_transpose(i, p_i):
                """PE-transpose attn weights (f32r); psum->sbuf casts to
                bf16 on the scalar engine."""
                pT = ptpool.tile([P, NKS, P], BF16, tag="pT")
                for g in range(4):
                    pt_ps = psT.tile([P, 4, P], F32R, tag="ptps")
                    for j in range(4):
                        s = g * 4 + j
                        nc.tensor.transpose(
                            pt_ps[:, j, :], p_i[:, ds(s * P, P)], ident_r[:]
                        )
                    nc.scalar.copy(pT[:, ds(g * 4, 4), :], pt_ps[:])
                return pT

            def pv_out(i, pT, recip):
                po = psO.tile([P, C], F32, tag="po2")
                for s in range(NKS):
                    for h in range(2):
                        nc.tensor.matmul(
                            po[:, ds(h * 512, 512)],
                            lhsT=pT[:, s, :],
                            rhs=xx[:, s, ds(h * 512, 512)],
                            start=(s == 0),
                            stop=(s == NKS - 1),
                        )
                out_t = opool.tile([P, C], F32, tag="out")
                # on the DVE: ACT must stay clear so exp(i+1) starts the
                # moment negmax lands (it gates all next-stripe PE work)
                nc.vector.tensor_scalar_mul(out_t[:], po[:], recip[:])
                nc.sync.dma_start(o_d[ts(i, P), :], out_t[:])

            # software pipeline; scores(i+1) is emitted with raised
            # scheduler priority so all its bank matmuls (and the save/
            # mask/rowmax chain they feed) land before pvT(i)/out(i) --
            # otherwise the scheduler defers the last bank past out(i) and
            # the [mms->save->mask->max->exp] tail lands on the critical
            # path at every stripe boundary
            sc = scores(0)
            sm = softmax(0, *sc)
            for i in range(NT):
                if i + 1 < NT:
                    with tc.high_priority(offset=100):
                        nxt_sc = scores(i + 1)
                else:
                    nxt_sc = None
                pT = pv_transpose(i, sm[0])
                nxt_sm = softmax(i + 1, *nxt_sc) if nxt_sc else None
                pv_out(i, pT, sm[1])
                sm = nxt_sm


def _split_sync_waits(nc, limit: int = 1):
    """The walrus build in this container rejects instructions with more than
    one sync-wait command.  Hoist excess waits onto preceding single-wait
    NoOps on the same engine (waits execute in order before the original
    instruction, so semantics are preserved)."""
    n_new = 0
    for fn in nc.m.functions:
        for blk in fn.blocks:
            new_insts = []
            for inst in blk.instructions:
                si = inst.sync_info
                if si and si.on_wait and len(si.on_wait) > limit:
                    waits = list(si.on_wait)
                    extra, keep = waits[:-limit], waits[-limit:]
                    for w in extra:
                        nop = mybir.InstNoOp(
                            name=f"{inst.name}-wsplit{n_new}", ins=[], outs=[]
                        )
                        n_new += 1
                        nop.engine = inst.engine
                        nop.sync_info = mybir.SyncInfo(on_wait=[w], on_update=[])
                        new_insts.append(nop)
                    si.on_wait[:] = keep
                new_insts.append(inst)
            blk.instructions[:] = new_insts
    return n_new


_NC_CACHE = None


def _get_nc():
    global _NC_CACHE
    if _NC_CACHE is None:
        nc = build_bass()
        _split_sync_waits(nc, limit=1)
        _NC_CACHE = nc
    return _NC_CACHE


def run(inputs: dict, trace: bool = False, tmpdir: str | None = None):
    """Run on 8 NeuronCores; returns (out [B,T,C] f32, BassKernelResults)."""
    nc = _get_nc()
    x = np.ascontiguousarray(np.asarray(inputs["x"], dtype=np.float32))
    mask = np.asarray(inputs["mask"])
    if mask.dtype != np.uint8:
        mask = mask.astype(np.uint8)
    mask = np.ascontiguousarray(mask)
    w = np.ascontiguousarray(np.asarray(inputs["W"], dtype=np.float32))
    in_maps = [
        {"x": x[b], "mask": mask[b], "W": w} for b in range(B)
    ]
    res = bass_utils.run_bass_kernel_spmd(
        nc,
        in_maps,
        core_ids=list(range(B)),
        trace=trace,
        tmpdir=tmpdir,
    )
    out = np.stack([res.results[b]["out"] for b in range(B)], axis=0)
    return out, res


def kernel(**inputs) -> np.ndarray:
    out, _ = run(inputs, trace=False)
    return out


# revision 25
# speedup vs baseline: 1.5174x; 1.2248x over previous
"""CrossLinear attention kernel for Trainium2 (8 NeuronCores, data-parallel over batch).

Computes, per batch element b:
    scores = x_b @ x_b^T            [T, T]
    scores[mask] = -inf
    attn = softmax(scores, axis=-1)
    xx = x_b @ W                    [T, C]
    out_b = attn @ xx               [T, C]

with B=8, T=2048, C=1024 (fp32).  One batch element per NeuronCore.

Design notes (v2 — symmetric scores):
  - All big matmuls use float32r operands -> 1 cycle/row on the PE when the
    moving dim is >=256 (plain fp32 is 4 cycles/row).  End-to-end rel err
    stays ~1e-3.
  - scores = x x^T is SYMMETRIC: stripe i only matmuls blocks (i, j>=i);
    blocks (i, j<i) are PE-transposed (1.5 cyc/row, ~5x cheaper than the
    matmul they replace) from raw (pre-mask) 128x128 blocks saved by
    earlier stripes.  Raw blocks are copied PSUM->SBUF on the scalar
    engine before the DVE mask-fill of the same PSUM region; a 64-slot
    interval-colored save pool (exact peak occupancy) holds them.
  - x is transposed once on the PE into xT [C, T] (f32r input via ACT cast
    so the transposes run 1.5 cyc/row); both score-matmul operands and the
    x@W lhsT come from xT.
  - The attn @ xx GEMM runs in bf16: xx is stored bf16 and the attn
    transposes (f32r on the PE) are cast to bf16 on the psum->sbuf copy.
    PSUM accumulates fp32, so only input rounding (~0.2%) is added.
  - Masking via copy_predicated (mask u8, -1e9 fill) per 512-col bank,
    interleaved with the score work; softmax: per-bank DVE row max ->
    combine (negated) -> ACT exp with per-partition bias and accumulated
    row sum -> 1/sum folded into the final output scale.
  - Phase ordering: x DMAs are issued before W so the PE starts early; the
    x-transposes and x@W matmuls interleave; scores(0) is emitted right
    after the last x@W matmul so the PE never drains at the phase
    boundary.  PSUM pools are staged (psA+psXX = 6 banks, then psS joins
    at 4+4 = 8, then psT/psO reuse psXX's banks).
  - This container's walrus accepts at most ONE sync-wait per instruction;
    _split_sync_waits hoists extras onto single-wait NoOps.
"""

import sys

if "/opt/trn_rl_repo" not in sys.path:
    sys.path.insert(0, "/opt/trn_rl_repo")

from contextlib import ExitStack

import numpy as np

import concourse.bass as bass
import concourse.mybir as mybir
import concourse.tile as tile
from concourse import bass_utils
from concourse.bass import ds, ts
from concourse.masks import make_identity

B, T, C = 8, 2048, 1024
P = 128                 # partition block
NT = T // P             # 16 row blocks (stripes)
NKC = C // P            # 8 contraction chunks over C
NKS = T // P            # 16 contraction chunks over T (for attn @ xx)
NSLOTS = 64             # saved raw score blocks (exact interval-coloring peak)
NEG_BIG = -1.0e9

F32 = mybir.dt.float32
F32R = mybir.dt.float32r
BF16 = mybir.dt.bfloat16
U8 = mybir.dt.uint8


def _span_chunks(start_col: int):
    """Bank-aligned (512-col) chunk widths covering [start_col, T)."""
    chunks = []
    col = start_col
    while col < T:
        bank_end = (col // 512 + 1) * 512
        w = min(bank_end, T) - col
        chunks.append((col, w))
        col += w
    return chunks


def build_bass():
    nc = bass.Bass(
        trn_type="TRN2",
        target_bir_lowering=False,
        debug=False,
        enable_asserts=False,
        num_devices=8,
    )
    x_d = nc.dram_tensor("x", [T, C], F32, kind="ExternalInput").ap()
    m_d = nc.dram_tensor("mask", [T, T], U8, kind="ExternalInput").ap()
    w_d = nc.dram_tensor("W", [C, C], F32, kind="ExternalInput").ap()
    o_d = nc.dram_tensor("out", [T, C], F32, kind="ExternalOutput").ap()

    with tile.TileContext(nc) as tc:
        _kernel_body(nc, tc, x_d, m_d, w_d, o_d)
    return nc


def _kernel_body(nc, tc, x_d, m_d, w_d, o_d):
    with ExitStack() as big:
        const = big.enter_context(tc.tile_pool(name="const", bufs=1))
        ident = const.tile([P, P], F32)
        make_identity(nc, ident[:])
        ident_r = const.tile([P, P], F32R)
        nc.vector.tensor_copy(ident_r[:], ident[:])
        negbig = const.tile([P, 512], F32)
        nc.gpsimd.memset(negbig[:], NEG_BIG)

        persist = big.enter_context(tc.tile_pool(name="persist", bufs=1))
        xT = persist.tile([P, NKC, T], F32R)   # xT[p, k, t] = x[t, k*128+p]
        xx = persist.tile([P, NT, C], BF16)    # xx[p, i, d] = (x@W)[i*128+p, d]
        # raw upper-triangle score blocks, producer-contiguous: stripe i's
        # saved span (cols 128(i+1)..T) lives at block offset sbase[i], so
        # one batched ACT copy per 512-col bank saves it.  block (j,i)
        # (consumed by stripe j>i) sits at sbase[i] + (j-i-1).
        sbase = [0]
        for i in range(1, NT):
            sbase.append(sbase[-1] + (NT - 1 - i + 1))
        save = persist.tile([P, sbase[-1] * P], F32R)

        def save_cols(j, i):
            """column range in `save` holding raw block (j, i), j > i."""
            return ds((sbase[i] + (j - i - 1)) * P, P)

        # W (f32r, 32KB/partition) is dead once the last x@W matmul ran and
        # `save` is first written in phase B -- alias W onto save's first
        # 8K columns; the framework's WAR tracking orders the reuse.
        def wv(k, h):
            return save[:, ds(k * C + h * 512, 512)]

        # ---- Phase A: transpose x into xT; compute xx = x @ W ----
        # f32r is raw fp32 bytes (the TF32-style rounding happens inside the
        # PE), so x and W DMA straight into f32r tiles via bitcast views --
        # no cast instructions, and the PE can start as soon as x[0] lands.
        phA = ExitStack()
        xload = phA.enter_context(tc.tile_pool(name="xload", bufs=3))
        psXX = phA.enter_context(tc.tile_pool(name="psXX", bufs=2, space="PSUM"))
        psA = phA.enter_context(tc.tile_pool(name="psA", bufs=2, space="PSUM"))

        xts = {}

        def xdma(i):
            t = xload.tile([P, C], F32R, tag="xt")
            nc.sync.dma_start(t[:], x_d[ts(i, P), :].bitcast(F32R))
            xts[i] = t

        def xpose_c(i):
            xr = xts.pop(i)
            for g in range(2):
                pt = psA.tile([P, 4, P], F32R, tag="pt")
                for j in range(4):
                    k = g * 4 + j
                    nc.tensor.transpose(
                        pt[:, j, :], xr[:, ds(k * P, P)], ident_r[:]
                    )
                nc.vector.tensor_copy(
                    xT[:, ds(g * 4, 4), ds(i * P, P)], pt[:]
                )

        def xxmm(i):
            po = psXX.tile([P, C], F32, tag="po1")
            for k in range(NKC):
                for h in range(2):
                    nc.tensor.matmul(
                        po[:, ds(h * 512, 512)],
                        lhsT=xT[:, k, ds(i * P, P)],
                        rhs=wv(k, h),
                        start=(k == 0),
                        stop=(k == NKC - 1),
                    )
            nc.vector.tensor_copy(xx[:, i, :], po[:])

        # x DMAs first so the PE can start early; W loads overlap.
        xdma(0)
        xdma(1)
        xdma(2)
        for k in range(NKC):
            nc.sync.dma_start(
                save[:, ds(k * C, C)], w_d[ts(k, P), :].bitcast(F32R)
            )

        xpose_c(0)
        for i in range(1, NT):
            xpose_c(i)
            if i + 2 < NT:
                xdma(i + 2)
            xxmm(i - 1)
        xxmm(NT - 1)

        # release phase-A SBUF + PSUM so phase B can reuse the space; the
        # PE queue still flows straight from xx(15) into scores(0)
        phA.close()

        # ---- Phase B: attention main loop over row stripes ----
        with tc.tile_pool(name="maskp", bufs=2) as maskp, \
             tc.tile_pool(name="ppool", bufs=2) as ppool, \
             tc.tile_pool(name="ptpool", bufs=2) as ptpool, \
             tc.tile_pool(name="opool", bufs=2) as opool, \
             tc.tile_pool(name="stats", bufs=4) as stats, \
             tc.tile_pool(name="psS", bufs=1, space="PSUM") as psS, \
             tc.tile_pool(name="psT", bufs=2, space="PSUM") as psT, \
             tc.tile_pool(name="psO", bufs=1, space="PSUM") as psO:

            def scores(i):
                """Score stripe i.  Each 512-col bank lives in its OWN psum
                tile so the per-bank [transposes/matmuls -> raw save ->
                mask-fill -> row max -> exp] chains carry no false
                dependencies on each other -- a single [P,T] tile made the
                scheduler serialize bank n+1's matmuls behind bank n's DVE
                ops."""
                msk = maskp.tile([P, T], U8, tag="mask")
                nc.sync.dma_start(msk[:], m_d[ts(i, P), :])
                ps = [
                    psS.tile([P, 512], F32, tag=f"sc{n}", name=f"sc{n}")
                    for n in range(4)
                ]
                maxpart = stats.tile([P, 4], F32, tag="maxpart")
                for n in range(4):
                    sl = ds(n * 512, 512)
                    b0, b1 = n * 4, n * 4 + 4   # 128-col blocks of this bank
                    # left part: PE-transpose saved raw blocks
                    for j in range(b0, min(b1, i)):
                        nc.tensor.transpose(
                            ps[n][:, ds((j - b0) * P, P)].bitcast(F32R),
                            save[:, save_cols(i, j)],
                            ident_r[:],
                        )
                    # right part: one bank-aligned matmul chunk
                    mm0 = max(b0 * P, i * P)
                    w = (b1 * P) - mm0
                    if w > 0:
                        msl = ds(mm0, w)
                        for k in range(NKC):
                            nc.tensor.matmul(
                                ps[n][:, ds(mm0 - b0 * P, w)],
                                lhsT=xT[:, k, ds(i * P, P)],
                                rhs=xT[:, k, msl],
                                start=(k == 0),
                                stop=(k == NKC - 1),
                            )
                    # batched raw save of this bank's j>i blocks (pre-mask)
                    s0 = max(b0, i + 1)
                    if s0 < b1:
                        nw = (b1 - s0) * P
                        nc.scalar.copy(
                            save[:, ds((sbase[i] + s0 - i - 1) * P, nw)],
                            ps[n][:, ds((s0 - b0) * P, nw)],
                        )
                    # mask + row max (DVE)
                    nc.vector.copy_predicated(ps[n][:], msk[:, sl], negbig[:])
                    nc.vector.reduce_max(
                        maxpart[:, ds(n, 1)], ps[n][:],
                        axis=mybir.AxisListType.X,
                    )
                return ps, maxpart

            def softmax(i, ps, maxpart):
                negmax = stats.tile([P, 1], F32, tag="negmax")
                nc.vector.reduce_max(
                    negmax[:], maxpart[:], axis=mybir.AxisListType.X, negate=True
                )
                # per-bank exp into per-bank SBUF tiles (matching the psum
                # split); rowsum accumulated per bank then combined
                rowsums = stats.tile([P, 4], F32, tag="rowsums")
                p_i = [
                    ppool.tile([P, 512], F32R, tag=f"p{g}", name=f"p{g}")
                    for g in range(4)
                ]
                for g in range(4):
                    nc.scalar.activation(
                        p_i[g][:],
                        ps[g][:],
                        mybir.ActivationFunctionType.Exp,
                        bias=negmax[:],
                        scale=1.0,
                        accum_out=rowsums[:, ds(g, 1)],
                    )
                rowsum = stats.tile([P, 1], F32, tag="rowsum")
                nc.vector.reduce_sum(
                    rowsum[:], rowsums[:], axis=mybir.AxisListType.X
                )
                recip = stats.tile([P, 1], F32, tag="recip")
                nc.vector.reciprocal(recip[:], rowsum[:])
                return p_i, recip

            def pv_transpose(i, p_i):
                """PE-transpose attn weights (f32r); psum->sbuf casts to
                bf16 on the scalar engine.  Group g only touches exp part
                g's output tile."""
                pT = ptpool.tile([P, NKS, P], BF16, tag="pT")
                for g in range(4):
                    pt_ps = psT.tile([P, 4, P], F32R, tag="ptps")
                    for j in range(4):
                        nc.tensor.transpose(
                            pt_ps[:, j, :], p_i[g][:, ds(j * P, P)], ident_r[:]
                        )
                    nc.scalar.copy(pT[:, ds(g * 4, 4), :], pt_ps[:])
                return pT

            def pv_out(i, pT, recip):
                po = psO.tile([P, C], F32, tag="po2")
                for s in range(NKS):
                    for h in range(2):
                        nc.tensor.matmul(
                            po[:, ds(h * 512, 512)],
                            lhsT=pT[:, s, :],
                            rhs=xx[:, s, ds(h * 512, 512)],
                            start=(s == 0),
                            stop=(s == NKS - 1),
                        )
                out_t = opool.tile([P, C], F32, tag="out")
                # on the DVE: ACT must stay clear so exp(i+1) starts the
                # moment negmax lands (it gates all next-stripe PE work)
                nc.vector.tensor_scalar_mul(out_t[:], po[:], recip[:])
                nc.sync.dma_start(o_d[ts(i, P), :], out_t[:])

            # software pipeline; scores(i+1) is emitted with raised
            # scheduler priority so all its bank matmuls (and the save/
            # mask/rowmax chain they feed) land before pvT(i)/out(i) --
            # otherwise the scheduler defers the last bank past out(i) and
            # the [mms->save->mask->max->exp] tail lands on the critical
            # path at every stripe boundary
            sc = scores(0)
            sm = softmax(0, *sc)
            for i in range(NT):
                if i + 1 < NT:
                    with tc.high_priority(offset=100):
                        nxt_sc = scores(i + 1)
                else:
                    nxt_sc = None
                pT = pv_transpose(i, sm[0])
                nxt_sm = softmax(i + 1, *nxt_sc) if nxt_sc else None
                pv_out(i, pT, sm[1])
                sm = nxt_sm


def _split_sync_waits(nc, limit: int = 1):
    """The walrus build in this container rejects instructions with more than
    one sync-wait command.  Hoist excess waits onto preceding single-wait
    NoOps on the same engine (waits execute in order before the original
    instruction, so semantics are preserved)."""
    n_new = 0
    for fn in nc.m.functions:
        for blk in fn.blocks:
            new_insts = []
            for inst in blk.instructions:
                si = inst.sync_info
                if si and si.on_wait and len(si.on_wait) > limit:
                    waits = list(si.on_wait)
                    extra, keep = waits[:-limit], waits[-limit:]
                    for w in extra:
                        nop = mybir.InstNoOp(
                            name=f"{inst.name}-wsplit{n_new}", ins=[], outs=[]
                        )
                        n_new += 1
                        nop.engine = inst.engine
                        nop.sync_info = mybir.SyncInfo(on_wait=[w], on_update=[])
                        new_insts.append(nop)
                    si.on_wait[:] = keep
                new_insts.append(inst)
            blk.instructions[:] = new_insts
    return n_new


_NC_CACHE = None


def _get_nc():
    global _NC_CACHE
    if _NC_CACHE is None:
        nc = build_bass()
        _split_sync_waits(nc, limit=1)
        _NC_CACHE = nc
    return _NC_CACHE


def run(inputs: dict, trace: bool = False, tmpdir: str | None = None):
    """Run on 8 NeuronCores; returns (out [B,T,C] f32, BassKernelResults)."""
    nc = _get_nc()
    x = np.ascontiguousarray(np.asarray(inputs["x"], dtype=np.float32))
    mask = np.asarray(inputs["mask"])
    if mask.dtype != np.uint8:
        mask = mask.astype(np.uint8)
    mask = np.ascontiguousarray(mask)
    w = np.ascontiguousarray(np.asarray(inputs["W"], dtype=np.float32))
    in_maps = [
        {"x": x[b], "mask": mask[b], "W": w} for b in range(B)
    ]
    res = bass_utils.run_bass_kernel_spmd(
        nc,
        in_maps,
        core_ids=list(range(B)),
        trace=trace,
        tmpdir=tmpdir,
    )
    out = np.stack([res.results[b]["out"] for b in range(B)], axis=0)
    return out, res


def kernel(**inputs) -> np.ndarray:
    out, _ = run(inputs, trace=False)
    return out


# revision 30
# speedup vs baseline: 1.5246x; 1.0047x over previous
"""CrossLinear attention kernel for Trainium2 (8 NeuronCores, data-parallel over batch).

Computes, per batch element b:
    scores = x_b @ x_b^T            [T, T]
    scores[mask] = -inf
    attn = softmax(scores, axis=-1)
    xx = x_b @ W                    [T, C]
    out_b = attn @ xx               [T, C]

with B=8, T=2048, C=1024 (fp32).  One batch element per NeuronCore.

Design notes (v2 — symmetric scores):
  - All big matmuls use float32r operands -> 1 cycle/row on the PE when the
    moving dim is >=256 (plain fp32 is 4 cycles/row).  End-to-end rel err
    stays ~1e-3.
  - scores = x x^T is SYMMETRIC: stripe i only matmuls blocks (i, j>=i);
    blocks (i, j<i) are PE-transposed (1.5 cyc/row, ~5x cheaper than the
    matmul they replace) from raw (pre-mask) 128x128 blocks saved by
    earlier stripes.  Raw blocks are copied PSUM->SBUF on the scalar
    engine before the DVE mask-fill of the same PSUM region; a 64-slot
    interval-colored save pool (exact peak occupancy) holds them.
  - x is transposed once on the PE into xT [C, T] (f32r input via ACT cast
    so the transposes run 1.5 cyc/row); both score-matmul operands and the
    x@W lhsT come from xT.
  - The attn @ xx GEMM runs in bf16: xx is stored bf16 and the attn
    transposes (f32r on the PE) are cast to bf16 on the psum->sbuf copy.
    PSUM accumulates fp32, so only input rounding (~0.2%) is added.
  - Masking via copy_predicated (mask u8, -1e9 fill) per 512-col bank,
    interleaved with the score work; softmax: per-bank DVE row max ->
    combine (negated) -> ACT exp with per-partition bias and accumulated
    row sum -> 1/sum folded into the final output scale.
  - Phase ordering: x DMAs are issued before W so the PE starts early; the
    x-transposes and x@W matmuls interleave; scores(0) is emitted right
    after the last x@W matmul so the PE never drains at the phase
    boundary.  PSUM pools are staged (psA+psXX = 6 banks, then psS joins
    at 4+4 = 8, then psT/psO reuse psXX's banks).
  - This container's walrus accepts at most ONE sync-wait per instruction;
    _split_sync_waits hoists extras onto single-wait NoOps.
"""

import sys

if "/opt/trn_rl_repo" not in sys.path:
    sys.path.insert(0, "/opt/trn_rl_repo")

from contextlib import ExitStack

import numpy as np

import concourse.bass as bass
import concourse.mybir as mybir
import concourse.tile as tile
from concourse import bass_utils
from concourse.bass import ds, ts
from concourse.masks import make_identity

B, T, C = 8, 2048, 1024
P = 128                 # partition block
NT = T // P             # 16 row blocks (stripes)
NKC = C // P            # 8 contraction chunks over C
NKS = T // P            # 16 contraction chunks over T (for attn @ xx)
NSLOTS = 64             # saved raw score blocks (exact interval-coloring peak)
NEG_BIG = -1.0e9

F32 = mybir.dt.float32
F32R = mybir.dt.float32r
BF16 = mybir.dt.bfloat16
U8 = mybir.dt.uint8


def _span_chunks(start_col: int):
    """Bank-aligned (512-col) chunk widths covering [start_col, T)."""
    chunks = []
    col = start_col
    while col < T:
        bank_end = (col // 512 + 1) * 512
        w = min(bank_end, T) - col
        chunks.append((col, w))
        col += w
    return chunks


def build_bass():
    nc = bass.Bass(
        trn_type="TRN2",
        target_bir_lowering=False,
        debug=False,
        enable_asserts=False,
        num_devices=8,
    )
    x_d = nc.dram_tensor("x", [T, C], F32, kind="ExternalInput").ap()
    m_d = nc.dram_tensor("mask", [T, T], U8, kind="ExternalInput").ap()
    w_d = nc.dram_tensor("W", [C, C], F32, kind="ExternalInput").ap()
    o_d = nc.dram_tensor("out", [T, C], F32, kind="ExternalOutput").ap()

    with tile.TileContext(nc) as tc:
        _kernel_body(nc, tc, x_d, m_d, w_d, o_d)
    return nc


def _kernel_body(nc, tc, x_d, m_d, w_d, o_d):
    with ExitStack() as big:
        const = big.enter_context(tc.tile_pool(name="const", bufs=1))
        ident = const.tile([P, P], F32)
        make_identity(nc, ident[:])
        ident_r = const.tile([P, P], F32R)
        nc.vector.tensor_copy(ident_r[:], ident[:])
        ident_b = const.tile([P, P], BF16)
        nc.vector.tensor_copy(ident_b[:], ident[:])
        negbig = const.tile([P, 512], F32)
        nc.gpsimd.memset(negbig[:], NEG_BIG)

        persist = big.enter_context(tc.tile_pool(name="persist", bufs=1))
        xT = persist.tile([P, NKC, T], F32R)   # xT[p, k, t] = x[t, k*128+p]
        xx = persist.tile([P, NT, C], BF16)    # xx[p, i, d] = (x@W)[i*128+p, d]
        # raw upper-triangle score blocks, producer-contiguous: stripe i's
        # saved span (cols 128(i+1)..T) lives at block offset sbase[i], so
        # one batched ACT copy per 512-col bank saves it.  block (j,i)
        # (consumed by stripe j>i) sits at sbase[i] + (j-i-1).
        sbase = [0]
        for i in range(1, NT):
            sbase.append(sbase[-1] + (NT - 1 - i + 1))
        save = persist.tile([P, sbase[-1] * P], F32R)

        def save_cols(j, i):
            """column range in `save` holding raw block (j, i), j > i."""
            return ds((sbase[i] + (j - i - 1)) * P, P)

        # W (f32r, 32KB/partition) is dead once the last x@W matmul ran and
        # `save` is first written in phase B -- alias W onto save's first
        # 8K columns; the framework's WAR tracking orders the reuse.
        def wv(k, h):
            return save[:, ds(k * C + h * 512, 512)]

        # ---- Phase A: transpose x into xT; compute xx = x @ W ----
        # f32r is raw fp32 bytes (the TF32-style rounding happens inside the
        # PE), so x and W DMA straight into f32r tiles via bitcast views --
        # no cast instructions, and the PE can start as soon as x[0] lands.
        phA = ExitStack()
        xload = phA.enter_context(tc.tile_pool(name="xload", bufs=3))
        psXX = phA.enter_context(tc.tile_pool(name="psXX", bufs=2, space="PSUM"))
        psA = phA.enter_context(tc.tile_pool(name="psA", bufs=2, space="PSUM"))

        xts = {}

        def xdma(i, split=False):
            t = xload.tile([P, C], F32R, tag="xt")
            if split:
                # halves land on separate DMA queues, so the first
                # transposes can start ~2us sooner at kernel start
                for h in range(2):
                    nc.sync.dma_start(
                        t[:, ds(h * 512, 512)],
                        x_d[ts(i, P), ds(h * 512, 512)].bitcast(F32R),
                    )
            else:
                nc.sync.dma_start(t[:], x_d[ts(i, P), :].bitcast(F32R))
            xts[i] = t

        def xpose_c(i):
            xr = xts.pop(i)
            for g in range(2):
                pt = psA.tile([P, 4, P], F32R, tag="pt")
                for j in range(4):
                    k = g * 4 + j
                    nc.tensor.transpose(
                        pt[:, j, :], xr[:, ds(k * P, P)], ident_r[:]
                    )
                nc.vector.tensor_copy(
                    xT[:, ds(g * 4, 4), ds(i * P, P)], pt[:]
                )

        def xxmm(i):
            po = psXX.tile([P, C], F32, tag="po1")
            for k in range(NKC):
                for h in range(2):
                    nc.tensor.matmul(
                        po[:, ds(h * 512, 512)],
                        lhsT=xT[:, k, ds(i * P, P)],
                        rhs=wv(k, h),
                        start=(k == 0),
                        stop=(k == NKC - 1),
                    )
            nc.vector.tensor_copy(xx[:, i, :], po[:])

        # x DMAs first so the PE can start early; W loads overlap.
        xdma(0, split=True)
        xdma(1)
        xdma(2)
        for k in range(NKC):
            nc.sync.dma_start(
                save[:, ds(k * C, C)], w_d[ts(k, P), :].bitcast(F32R)
            )

        xpose_c(0)
        for i in range(1, NT):
            xpose_c(i)
            if i + 2 < NT:
                xdma(i + 2)
            xxmm(i - 1)
        xxmm(NT - 1)

        # release phase-A SBUF + PSUM so phase B can reuse the space; the
        # PE queue still flows straight from xx(15) into scores(0)
        phA.close()

        # ---- Phase B: attention main loop over row stripes ----
        with tc.tile_pool(name="maskp", bufs=2) as maskp, \
             tc.tile_pool(name="ppool", bufs=2) as ppool, \
             tc.tile_pool(name="ptpool", bufs=2) as ptpool, \
             tc.tile_pool(name="opool", bufs=2) as opool, \
             tc.tile_pool(name="stats", bufs=4) as stats, \
             tc.tile_pool(name="psS", bufs=1, space="PSUM") as psS, \
             tc.tile_pool(name="psT", bufs=2, space="PSUM") as psT, \
             tc.tile_pool(name="psO", bufs=1, space="PSUM") as psO:

            def scores(i):
                """Score stripe i.  Each 512-col bank lives in its OWN psum
                tile so the per-bank [transposes/matmuls -> raw save ->
                mask-fill -> row max -> exp] chains carry no false
                dependencies on each other -- a single [P,T] tile made the
                scheduler serialize bank n+1's matmuls behind bank n's DVE
                ops."""
                msk = maskp.tile([P, T], U8, tag="mask")
                nc.sync.dma_start(msk[:], m_d[ts(i, P), :])
                ps = [
                    psS.tile([P, 512], F32, tag=f"sc{n}", name=f"sc{n}")
                    for n in range(4)
                ]
                maxpart = stats.tile([P, 4], F32, tag="maxpart")
                for n in range(4):
                    sl = ds(n * 512, 512)
                    b0, b1 = n * 4, n * 4 + 4   # 128-col blocks of this bank
                    # left part: PE-transpose saved raw blocks
                    for j in range(b0, min(b1, i)):
                        nc.tensor.transpose(
                            ps[n][:, ds((j - b0) * P, P)].bitcast(F32R),
                            save[:, save_cols(i, j)],
                            ident_r[:],
                        )
                    # right part: one bank-aligned matmul chunk
                    mm0 = max(b0 * P, i * P)
                    w = (b1 * P) - mm0
                    if w > 0:
                        msl = ds(mm0, w)
                        for k in range(NKC):
                            nc.tensor.matmul(
                                ps[n][:, ds(mm0 - b0 * P, w)],
                                lhsT=xT[:, k, ds(i * P, P)],
                                rhs=xT[:, k, msl],
                                start=(k == 0),
                                stop=(k == NKC - 1),
                            )
                    # batched raw save of this bank's j>i blocks (pre-mask)
                    s0 = max(b0, i + 1)
                    if s0 < b1:
                        nw = (b1 - s0) * P
                        nc.scalar.copy(
                            save[:, ds((sbase[i] + s0 - i - 1) * P, nw)],
                            ps[n][:, ds((s0 - b0) * P, nw)],
                        )
                    # mask + row max (DVE)
                    nc.vector.copy_predicated(ps[n][:], msk[:, sl], negbig[:])
                    nc.vector.reduce_max(
                        maxpart[:, ds(n, 1)], ps[n][:],
                        axis=mybir.AxisListType.X,
                    )
                return ps, maxpart

            def softmax(i, ps, maxpart):
                negmax = stats.tile([P, 1], F32, tag="negmax")
                nc.vector.reduce_max(
                    negmax[:], maxpart[:], axis=mybir.AxisListType.X, negate=True
                )
                # per-bank exp into per-bank SBUF tiles (matching the psum
                # split); rowsum accumulated per bank then combined.  bf16
                # output: the attn transposes then run 1 cyc/row (vs 1.5
                # f32r) and feed the bf16 attn @ xx GEMM directly.
                rowsums = stats.tile([P, 4], F32, tag="rowsums")
                p_i = [
                    ppool.tile([P, 512], BF16, tag=f"p{g}", name=f"p{g}")
                    for g in range(4)
                ]
                for g in range(4):
                    nc.scalar.activation(
                        p_i[g][:],
                        ps[g][:],
                        mybir.ActivationFunctionType.Exp,
                        bias=negmax[:],
                        scale=1.0,
                        accum_out=rowsums[:, ds(g, 1)],
                    )
                rowsum = stats.tile([P, 1], F32, tag="rowsum")
                nc.vector.reduce_sum(
                    rowsum[:], rowsums[:], axis=mybir.AxisListType.X
                )
                recip = stats.tile([P, 1], F32, tag="recip")
                nc.vector.reciprocal(recip[:], rowsum[:])
                return p_i, recip

            def pv_transpose(i, p_i):
                """PE-transpose attn weights (f32r); psum->sbuf casts to
                bf16 on the scalar engine.  Group g only touches exp part
                g's output tile."""
                pT = ptpool.tile([P, NKS, P], BF16, tag="pT")
                for g in range(4):
                    pt_ps = psT.tile([P, 4, P], BF16, tag="ptps")
                    for j in range(4):
                        nc.tensor.transpose(
                            pt_ps[:, j, :], p_i[g][:, ds(j * P, P)], ident_b[:]
                        )
                    nc.scalar.copy(pT[:, ds(g * 4, 4), :], pt_ps[:])
                return pT

            def pv_out(i, pT, recip):
                po = psO.tile([P, C], F32, tag="po2")
                for s in range(NKS):
                    for h in range(2):
                        nc.tensor.matmul(
                            po[:, ds(h * 512, 512)],
                            lhsT=pT[:, s, :],
                            rhs=xx[:, s, ds(h * 512, 512)],
                            start=(s == 0),
                            stop=(s == NKS - 1),
                        )
                out_t = opool.tile([P, C], F32, tag="out")
                # on the DVE: ACT must stay clear so exp(i+1) starts the
                # moment negmax lands (it gates all next-stripe PE work)
                nc.vector.tensor_scalar_mul(out_t[:], po[:], recip[:])
                nc.sync.dma_start(o_d[ts(i, P), :], out_t[:])

            # software pipeline; scores(i+1) is emitted with raised
            # scheduler priority so all its bank matmuls (and the save/
            # mask/rowmax chain they feed) land before pvT(i)/out(i) --
            # otherwise the scheduler defers the last bank past out(i) and
            # the [mms->save->mask->max->exp] tail lands on the critical
            # path at every stripe boundary
            sc = scores(0)
            sm = softmax(0, *sc)
            for i in range(NT):
                if i + 1 < NT:
                    with tc.high_priority(offset=100):
                        nxt_sc = scores(i + 1)
                else:
                    nxt_sc = None
                pT = pv_transpose(i, sm[0])
                nxt_sm = softmax(i + 1, *nxt_sc) if nxt_sc else None
                pv_out(i, pT, sm[1])
                sm = nxt_sm


def _split_sync_waits(nc, limit: int = 1):
    """The walrus build in this container rejects instructions with more than
    one sync-wait command.  Hoist excess waits onto preceding single-wait
    NoOps on the same engine (waits execute in order before the original
    instruction, so semantics are preserved)."""
    n_new = 0
    for fn in nc.m.functions:
        for blk in fn.blocks:
            new_insts = []
            for inst in blk.instructions:
                si = inst.sync_info
                if si and si.on_wait and len(si.on_wait) > limit:
                    waits = list(si.on_wait)
                    extra, keep = waits[:-limit], waits[-limit:]
                    for w in extra:
                        nop = mybir.InstNoOp(
                            name=f"{inst.name}-wsplit{n_new}", ins=[], outs=[]
                        )
                        n_new += 1
                        nop.engine = inst.engine
                        nop.sync_info = mybir.SyncInfo(on_wait=[w], on_update=[])
                        new_insts.append(nop)
                    si.on_wait[:] = keep
                new_insts.append(inst)
            blk.instructions[:] = new_insts
    return n_new


_NC_CACHE = None


def _get_nc():
    global _NC_CACHE
    if _NC_CACHE is None:
        nc = build_bass()
        _split_sync_waits(nc, limit=1)
        _NC_CACHE = nc
    return _NC_CACHE


def run(inputs: dict, trace: bool = False, tmpdir: str | None = None):
    """Run on 8 NeuronCores; returns (out [B,T,C] f32, BassKernelResults)."""
    nc = _get_nc()
    x = np.ascontiguousarray(np.asarray(inputs["x"], dtype=np.float32))
    mask = np.asarray(inputs["mask"])
    if mask.dtype != np.uint8:
        mask = mask.astype(np.uint8)
    mask = np.ascontiguousarray(mask)
    w = np.ascontiguousarray(np.asarray(inputs["W"], dtype=np.float32))
    in_maps = [
        {"x": x[b], "mask": mask[b], "W": w} for b in range(B)
    ]
    res = bass_utils.run_bass_kernel_spmd(
        nc,
        in_maps,
        core_ids=list(range(B)),
        trace=trace,
        tmpdir=tmpdir,
    )
    out = np.stack([res.results[b]["out"] for b in range(B)], axis=0)
    return out, res


def kernel(**inputs) -> np.ndarray:
    out, _ = run(inputs, trace=False)
    return out
